# revision 39
# baseline (speedup 1.0000x reference)
"""Trainium2 Bass kernel for a single attention head.

reference computation (fp32):
    q = query @ Wq + bq ; k = key @ Wk + bk ; v = value @ Wv + bv
    out = softmax((q @ k^T) / 8) @ v

Sharding: 8 cores, core c -> (batch b = c//2, query-half h = c%2).
Each core computes attention for its 2048 query rows against the full 4096
keys/values of its batch.

Design (bf16 pipeline, host-transposed inputs; rel err ~7e-3 vs 2e-2 gate):
  - host supplies X^T slices in bf16 ([512, rows], c-major) so activations
    DMA straight into SBUF in the projection-ready layout: no PE transposes
    of X, no PSUM->SBUF staging copies, and half the HBM traffic of fp32.
    All weights are packed into one bf16 DMA, biases into one fp32 DMA.
  - q/k projections on PE (bf16): lhsT = W [c-chunk, d], rhs = X^T chunk;
    bias folded into the mandatory PSUM->SBUF copy (DVE tensor_scalar_add).
    Qp^T [64, 2048] is duplicated to partitions 64:128 - by a second
    matmul at tile_position col offset for the latency-critical head
    blocks, by a sync-queue SBUF DMA for the rest; Kp^T is stored
    dual-half (even j-chunks on partitions 0:64, odd on 64:128) by
    issuing separate even/odd matmuls whose outputs land at partition
    offsets 0/64 via tile_position - no partition-shift DMA.
  - V projects directly into natural layout by swapping matmul operands:
    stationary = X^T block [128c,128j], moving = Wv [128c,66] -> out
    [128j,66] accumulated over c-chunks (66-column streams, no PE
    re-transpose); its free-dim bias is added via a broadcast DVE
    tensor_add. Col 64 of the padded Wv/bv is all-ones so the PV matmul
    also produces the softmax denominator; col 65 is zero padding.
  - scores^T tiles: lhsT = Kp^T[half, j-chunk] [64,128], rhs = Qp^T
    [64, 1024-i-slice] -> S^T [128 j, i] in PSUM; the two K=64 matmuls of a
    j-chunk pair occupy PE row-groups 0:64 / 64:128 (tile_position row
    tiling); exp fused with the 1/8 scale on ScalarE, output bf16 (no
    max-subtraction: |scores/8| <= ~3 so fp32 exp is safe).
  - PV accumulates NATURALLY by swapping operands as well: stationary =
    P^T i-chunk [128j,128i], moving = v chunk [128j,66] -> po[:, ic, :66]
    += P @ V, i.e. 66-column streams (half the PE cycles of the moving-P^T
    form) and an output that needs no transpose. po is [128, 8, 128] fp32
    so each 66-wide slice stays inside one 2KB PSUM bank; `start` fires
    only on each bank's first slice because start resets the whole bank
    row (col 64 = denominator via the ones column).
  - epilogue: reciprocal of po[:, ic, 64] on DVE + scale straight from
    PSUM into the fp32 output tile - no transposes, no staging copies; the
    final epilogue splits across DVE and the (by then idle) ScalarE.
  - scheduling: phase A (prework + i-half 0) is PE-bound while phase B
    (i-half 1) is ACT-bound, so the PV of several ih0 pairs is deferred
    into phase B: their exp outputs stay live in the pt pool and drain
    while ih1 scores/exp keep ACT saturated through the transition. The
    first pair's scores/exps are split per 256-column slice (and q block 0
    is loaded as two 256-row pieces) so ACT starts as soon as the first
    quarter of q lands; in the final phase PV runs before the lookahead
    scores so the PV backlog is drained by the time the last exp retires.
"""

import sys

if "/opt/trn_rl_repo" not in sys.path:
    sys.path.insert(0, "/opt/trn_rl_repo")

from contextlib import ExitStack

import numpy as np
import ml_dtypes

import concourse.bass as bass
import concourse.tile as tile
from concourse import bacc, mybir
from concourse.bass_utils import run_bass_kernel_spmd

F32 = mybir.dt.float32
BF16 = mybir.dt.bfloat16
NP_BF16 = ml_dtypes.bfloat16

B, S, C, D = 4, 4096, 512, 64
D2 = D + 2          # v padded with [ones, zeros] cols
WW = 4 * (D + D + D2)  # packed weight row: 776 bf16 per partition
N_CORES = 8
SQ = S // 2          # query rows per core
NJ = S // 128        # 32 key chunks of 128 rows
NP_ = NJ // 2        # 16 chunk pairs
IH = SQ // 2         # 1024: i-half processed per PSUM residency
EXP = mybir.ActivationFunctionType.Exp
COPY = mybir.ActivationFunctionType.Copy

# ih0 pairs whose PV is deferred into phase B (spread so phase A's per-g
# PE load stays balanced against ACT)
DEFER = frozenset((3, 5, 7, 9, 11, 13, 15))

_CACHE = {}


def _emit(nc, tc, aps):
    qT_d, kT_d, vT_d, w_d, b_d, out_d = aps

    ctx = ExitStack()
    const = ctx.enter_context(tc.tile_pool(name="const", bufs=1))
    persist = ctx.enter_context(tc.tile_pool(name="persist", bufs=1))
    stage_p = ctx.enter_context(tc.tile_pool(name="stage", bufs=4))
    pt_p = ctx.enter_context(tc.tile_pool(name="pt", bufs=34))
    small_p = ctx.enter_context(tc.tile_pool(name="small", bufs=4))
    out_p = ctx.enter_context(tc.tile_pool(name="outp", bufs=2))
    # PSUM budget (8 banks): scratch 2x1 + st 2x2 + po 1x2 = 8
    pp_ps = ctx.enter_context(tc.tile_pool(name="ppps", bufs=2, space="PSUM"))
    st_ps = ctx.enter_context(tc.tile_pool(name="stps", bufs=2, space="PSUM"))
    po_ps = ctx.enter_context(tc.tile_pool(name="pops", bufs=1, space="PSUM"))

    # packed weights [wk | wq | wvp], wk first in its own small DMA so the
    # k0 activation transfer queues behind ~0.2us of weights, not the lot
    w_all = const.tile([128, WW], BF16)
    nc.sync.dma_start(w_all[:], w_d[:])
    wk_sb = w_all[:, : 4 * D].rearrange("p (cc d) -> p cc d", d=D)
    wq_sb = w_all[:, 4 * D : 8 * D].rearrange("p (cc d) -> p cc d", d=D)
    wvp_sb = w_all[:, 8 * D :].rearrange("p (cc d) -> p cc d", d=D2)
    b_all = const.tile([128, 3 + D2], F32)
    nc.sync.dma_start(b_all[:], b_d[:])
    bq2_sb = b_all[:, 0:1]       # q bias duplicated on both halves
    bk2_sb = b_all[:, 1:2]
    bvp_bc = b_all[:, 3 : 3 + D2]   # v bias broadcast across partitions

    qpt = persist.tile([128, SQ], BF16)      # Qp^T duplicated on both halves
    kpt = persist.tile([128, S // 2], BF16)  # Kp^T dual-half (even|odd chunks)
    v_sb = persist.tile([128, NJ, D2], BF16)  # v natural + ones col

    def load_block(x_d, g):
        """DMA 512 c x 512 rows of a host-transposed activation into SBUF.
        SWDGE (gpsimd): keeps engine HWDGE queues free; Pool is idle anyway."""
        stg = stage_p.tile([128, 4, 512], BF16, tag="stage")
        nc.gpsimd.dma_start(
            stg[:],
            x_d[:, g * 512 : (g + 1) * 512].rearrange("(cc p) r -> p cc r", p=128),
        )
        return stg

    def proj_q(g, dup_mm=False, piece=None):
        # piece=(i): quarter-width warm-start load (cols i*256:(i+1)*256).
        # dup_mm: materialize the partition-64:128 copy with a second matmul
        # (tile_position col offset) instead of the slow SBUF dup-DMA chain -
        # used for the head blocks where the dup is latency-critical.
        if piece is None:
            w, sl = 512, slice(g * 512, (g + 1) * 512)
            src = qT_d[:, g * 512 : (g + 1) * 512]
        else:
            w, sl = 256, slice(piece * 256, (piece + 1) * 256)
            src = qT_d[:, piece * 256 : (piece + 1) * 256]
        stg = stage_p.tile([128, 4, w], BF16, tag="stage" if w == 512 else "stgq")
        nc.gpsimd.dma_start(stg[:], src.rearrange("(cc p) r -> p cc r", p=128))
        pp = pp_ps.tile([128, 512], F32, tag="pp")
        for cc in range(4):
            nc.tensor.matmul(
                pp[:D, :w], wq_sb[:, cc, :], stg[:, cc, :],
                start=(cc == 0), stop=(cc == 3),
            )
        if dup_mm:
            for cc in range(4):
                nc.tensor.matmul(
                    pp[D:, :w], wq_sb[:, cc, :], stg[:, cc, :],
                    start=(cc == 0), stop=(cc == 3),
                    tile_position=(0, D),
                )
        nc.vector.tensor_scalar_add(qpt[:D, sl], pp[:D, :w], bq2_sb[:D, :])
        if dup_mm:
            nc.vector.tensor_scalar_add(qpt[D:, sl], pp[D:, :w], bq2_sb[D:, :])
        else:
            nc.sync.dma_start(qpt[D:, sl], qpt[:D, sl])

    def proj_k(g):
        # block g covers j-chunks 4g..4g+3; even chunks project to output
        # partitions 0:64, odd to 64:128 (tile_position col offset), so the
        # bias-add writes kpt's dual-half layout directly.
        stg = load_block(kT_d, g)
        pp = pp_ps.tile([128, 512], F32, tag="pp")
        for half in range(2):
            for cc in range(4):
                rhs = stg[:, cc, :].rearrange("p (c n) -> p c n", n=128)[:, half::2, :]
                nc.tensor.matmul(
                    pp[half * D : (half + 1) * D, :256],
                    wk_sb[:, cc, :],
                    rhs,
                    start=(cc == 0), stop=(cc == 3),
                    tile_position=(0, half * D),
                )
        sl = slice(g * 256, (g + 1) * 256)
        nc.vector.tensor_scalar_add(kpt[:D, sl], pp[:D, :256], bk2_sb[:D, :])
        nc.vector.tensor_scalar_add(kpt[D:, sl], pp[D:, :256], bk2_sb[D:, :])

    def proj_v(g):
        # natural-layout projection: stationary = X^T block [128c,128j],
        # moving = Wv [128c,66] -> out [128j,66] accumulated over c-chunks.
        # 66-column streams instead of 512 + no PE re-transpose; the bias
        # (free-dim-varying here) is added via a broadcast tensor_add.
        stg = load_block(vT_d, g)
        for r in range(4):
            vp = pp_ps.tile([128, D2], F32, tag="pp")
            for cc in range(4):
                nc.tensor.matmul(
                    vp[:], stg[:, cc, r * 128 : (r + 1) * 128], wvp_sb[:, cc, :],
                    start=(cc == 0), stop=(cc == 3),
                )
            nc.vector.tensor_add(v_sb[:, g * 4 + r, :], vp[:], bvp_bc)

    def scores_exp(p, ih, nw=IH):
        # chunk pair p = chunks (2p, 2p+1): even on kpt rows 0:64, odd 64:128.
        # nw < 512 splits the matmul/exp into narrower column slices so the
        # first pair can start before the full q width has landed.
        sts = []
        for half in range(2):
            st = st_ps.tile([128, IH], F32, tag="st")
            pt = pt_p.tile([128, IH], BF16, tag="pt")
            mmw = min(nw, 512)           # matmul out must fit one PSUM bank
            for n in range(IH // mmw):
                lo = n * mmw
                nc.tensor.matmul(
                    st[:, lo : lo + mmw],
                    kpt[half * D : (half + 1) * D, p * 128 : (p + 1) * 128],
                    qpt[half * D : (half + 1) * D,
                        ih * IH + lo : ih * IH + lo + mmw],
                    tile_position=(half * D, 0),
                )
                if nw < IH:
                    nc.scalar.activation(
                        pt[:, lo : lo + nw], st[:, lo : lo + nw],
                        EXP, scale=0.125,
                    )
            if nw == IH:
                nc.scalar.activation(pt[:], st[:], EXP, scale=0.125)
            sts.append(pt)
        return sts

    def pv(p, po, sts, first, last):
        # natural accumulation: stationary = P^T i-chunk [128j,128i], moving
        # = v chunk [128j,66] -> po[:, ic, :66] += P_chunk @ V_chunk. 66-col
        # streams halve PV's PE cycles and the output needs no transpose.
        for half in range(2):
            for ic in range(IH // 128):
                # start resets the whole 2KB PSUM bank row, so only the first
                # slice per bank may carry it; later slices land on
                # has_written=0 cells and store rather than add
                nc.tensor.matmul(
                    po[:, ic, :D2],
                    sts[half][:, ic * 128 : (ic + 1) * 128],
                    v_sb[:, 2 * p + half, :],
                    start=(first and half == 0 and ic % 4 == 0),
                    stop=(last and half == 1),
                )

    def epilogue(ih, po, on_act):
        # po is already natural [i, (ic), d]: reciprocal of the denominator
        # column + scale, then DMA. on_act (final epilogue): the second half
        # runs on the by-then-idle ScalarE in parallel with DVE.
        osb = out_p.tile([128, IH // 128, D], F32, tag="osb")
        for ic in range(IH // 128):
            rs = small_p.tile([128, 1], F32, tag="rs")
            nc.vector.reciprocal(rs[:], po[:, ic, D : D + 1])
            if on_act and ic >= 4:
                nc.scalar.activation(osb[:, ic, :], po[:, ic, :D], COPY, scale=rs[:])
            else:
                nc.vector.tensor_scalar_mul(osb[:, ic, :], po[:, ic, :D], rs[:])
            if ic % 4 == 3:
                nc.sync.dma_start(
                    out_d[ih * IH + (ic - 3) * 128 : ih * IH + (ic + 1) * 128, :]
                    .rearrange("(t p) d -> p t d", p=128),
                    osb[:, ic - 3 : ic + 1, :],
                )

    # emission order = per-engine program order; sequence so the first exp
    # lands as early as possible and ACT never waits on deferrable PE work
    po0 = po_ps.tile([128, IH // 128, 128], F32, tag="po")
    deferred = []
    proj_k(0)
    proj_q(0, dup_mm=True, piece=0)
    proj_q(0, dup_mm=True, piece=1)
    proj_q(1, dup_mm=True)
    for g in range(8):
        sts0 = scores_exp(2 * g, 0, nw=(256 if g == 0 else IH))
        if g < 7:
            proj_k(g + 1)       # keep the k pipeline one block ahead
        proj_v(g)
        if 2 * g in DEFER:
            deferred.append((2 * g, sts0))
        else:
            pv(2 * g, po0, sts0, first=(g == 0), last=False)
        sts1 = scores_exp(2 * g + 1, 0)
        if g in (5, 6):
            proj_q(g - 3)       # q cols 1024:2048, needed only for i-half 1
        if 2 * g + 1 in DEFER:
            deferred.append((2 * g + 1, sts1))
        else:
            pv(2 * g + 1, po0, sts1, first=False, last=False)

    # drain deferred ih0 PVs while ih1 scores/exp keep ACT saturated
    exped = {}
    for i, (p0, sts) in enumerate(deferred):
        exped[i] = scores_exp(i, 1)
        pv(p0, po0, sts, first=False, last=(i == len(deferred) - 1))
    epilogue(0, po0, on_act=False)
    # po1 phase: lookahead pipeline — scores/exp stay ~AH pairs ahead of PV
    AH = len(deferred)
    po1 = po_ps.tile([128, IH // 128, 128], F32, tag="po")
    for p in range(NP_):
        pv(p, po1, exped.pop(p), first=(p == 0), last=(p == NP_ - 1))
        if p + AH < NP_:
            exped[p + AH] = scores_exp(p + AH, 1)
    epilogue(1, po1, on_act=True)
    ctx.close()


def _build(reps=1):
    nc = bacc.Bacc("TRN2", target_bir_lowering=False, debug=False, num_devices=N_CORES)
    aps = (
        nc.dram_tensor("qT", [C, SQ], BF16, kind="ExternalInput").ap(),
        nc.dram_tensor("kT", [C, S], BF16, kind="ExternalInput").ap(),
        nc.dram_tensor("vT", [C, S], BF16, kind="ExternalInput").ap(),
        nc.dram_tensor("w", [128, WW], BF16, kind="ExternalInput").ap(),
        nc.dram_tensor("b", [128, 3 + D2], F32, kind="ExternalInput").ap(),
        nc.dram_tensor("out", [SQ, D], F32, kind="ExternalOutput").ap(),
    )
    with tile.TileContext(nc) as tc:
        for _ in range(reps):
            _emit(nc, tc, aps)
    nc.compile()
    return nc


def get_nc():
    if "nc" not in _CACHE:
        _CACHE["nc"] = _build()
    return _CACHE["nc"]


def make_in_maps(query, key_, value, Wq, bq, Wk, bk, Wv, bv):
    query, key_, value, Wq, bq, Wk, bk, Wv, bv = (
        np.asarray(a, dtype=np.float32)
        for a in (query, key_, value, Wq, bq, Wk, bk, Wv, bv)
    )
    wvp = np.concatenate([Wv, np.zeros((C, 2), np.float32)], axis=1)
    # packed weights: per partition p, 4 c-chunks of [wq | wk | wvp] rows
    wcat = np.concatenate([Wk, Wq, wvp], axis=1)          # [512, 194]
    wr = wcat.reshape(4, 128, D + D + D2).transpose(1, 0, 2)   # [128, 4, 194]
    w = np.ascontiguousarray(
        np.concatenate(
            [wr[:, :, :D].reshape(128, 4 * D),          # wk, cc-major
             wr[:, :, D : 2 * D].reshape(128, 4 * D),   # wq
             wr[:, :, 2 * D :].reshape(128, 4 * D2)],   # wvp
            axis=1,
        ).astype(NP_BF16)
    )
    bvp = np.concatenate([bv, [1.0, 0.0], np.zeros(128 - D2, np.float32)])
    bvp_row = np.concatenate([bv, [1.0, 0.0]]).astype(np.float32)   # [66]
    b = np.ascontiguousarray(
        np.concatenate(
            [
                np.stack(
                    [
                        np.concatenate([bq, bq]),
                        np.concatenate([bk, bk]),
                        bvp,
                    ],
                    axis=1,
                ),
                np.tile(bvp_row, (128, 1)),
            ],
            axis=1,
        ).astype(np.float32)
    )
    kT = [np.ascontiguousarray(key_[b_].T.astype(NP_BF16)) for b_ in range(B)]
    vT = [np.ascontiguousarray(value[b_].T.astype(NP_BF16)) for b_ in range(B)]
    in_maps = []
    for c in range(N_CORES):
        b_, h = divmod(c, 2)
        in_maps.append(
            {
                "qT": np.ascontiguousarray(
                    query[b_, h * SQ : (h + 1) * SQ, :].T.astype(NP_BF16)
                ),
                "kT": kT[b_],
                "vT": vT[b_],
                "w": w,
                "b": b,
            }
        )
    return in_maps


def assemble(results):
    out = np.empty((B, S, D), np.float32)
    for c in range(N_CORES):
        b_, h = divmod(c, 2)
        out[b_, h * SQ : (h + 1) * SQ, :] = results[c]["out"]
    return out


def kernel(query=None, key_=None, value=None, Wq=None, bq=None, Wk=None,
           bk=None, Wv=None, bv=None, key=None, **_):
    if key_ is None:
        key_ = key          # spec names this input "key"; reference uses "key_"
    nc = get_nc()
    in_maps = make_in_maps(query, key_, value, Wq, bq, Wk, bk, Wv, bv)
    res = run_bass_kernel_spmd(nc, in_maps, list(range(N_CORES)))
    return assemble(res.results)
